# revision 29
# baseline (speedup 1.0000x reference)
"""NemotronH top-k MoE router on 8 Trainium2 NeuronCores (Bass/Tile).

Data-parallel over tokens: each of the 8 cores gets 2048 tokens.
Per core:
  - logits[128tok, 256e] = hidden @ weight.T at fp32-equivalent precision
    via an fp16 hi/lo 3-matmul decomposition (3 PE cycles/row instead of
    plain fp32's 4; error ~2^-24, at fp32's own rounding noise)
  - sigmoid (ACT)
  - DeepSeek-V3 style grouped top-k (noaux_tc) entirely with the DVE's
    native max/max_index/match_replace ops (ties resolve lowest-index
    first, exactly matching jax.lax.top_k)

Host side only reshapes/transposes/splits inputs (sharding prep) and
gathers outputs; all routing math runs on device.
"""

import sys
import numpy as np
from contextlib import ExitStack

for _p in ("/opt/trn_rl_repo", "/opt/pypackages"):
    if _p not in sys.path:
        sys.path.append(_p)

import concourse.bass as bass
import concourse.bacc as bacc
import concourse.tile as tile
import concourse.mybir as mybir
from concourse.bass_utils import run_bass_kernel_spmd

F32 = mybir.dt.float32
F16 = mybir.dt.float16
U32 = mybir.dt.uint32
ALU = mybir.AluOpType

# GEMM precision scheme:
#  "fp32"  : plain fp32 matmuls (4 cycles/row on the PE)
#  "fp16x3": x = hi + lo (fp16 hi, fp16 lo scaled by 2^12), w likewise;
#            logits = hi.hi + (hi.lo' + lo'.hi) * 2^-12, dropping the
#            lo.lo term (~2^-24 relative — at fp32's own noise floor).
#            3 matmuls at 1 cycle/row = 3 cycles/row total.
PRECISION = "fp16x3"
LO_SCALE = 4096.0          # 2^12
LO_INV = 1.0 / LO_SCALE

N_CORES = 8
TOKENS = 16384
HIDDEN = 4096
E = 256          # experts
G = 8            # groups
EPG = E // G     # experts per group (32)
TOPK_GROUP = 4
K = 8            # top-k experts
P = 128          # partitions
TPC = TOKENS // N_CORES   # tokens per core (2048)
KT = HIDDEN // P          # k tiles (32)
CHUNK = 2                 # token tiles per hidden DMA chunk
NEG = -1.0e30
ROUTED_SCALING = 2.5


def build_program(tpc: int = TPC, repeat: int = 1):
    """Build the SPMD Bass program (same on all cores).

    repeat > 1 re-runs the whole pipeline over the same data inside one
    NEFF — used only for wall-clock timing calibration (delta method).
    """
    nt = tpc // P  # token tiles per core
    nc = bacc.Bacc(
        "TRN2", target_bir_lowering=False, debug=False, num_devices=N_CORES
    )
    fp16 = PRECISION == "fp16x3"
    if fp16:
        hth = nc.dram_tensor("hth", [HIDDEN, tpc], F16, kind="ExternalInput").ap()
        htl = nc.dram_tensor("htl", [HIDDEN, tpc], F16, kind="ExternalInput").ap()
        wth = nc.dram_tensor("wth", [HIDDEN, E], F16, kind="ExternalInput").ap()
        wtl = nc.dram_tensor("wtl", [HIDDEN, E], F16, kind="ExternalInput").ap()
    else:
        ht = nc.dram_tensor("ht", [HIDDEN, tpc], F32, kind="ExternalInput").ap()
        wt = nc.dram_tensor("wt", [HIDDEN, E], F32, kind="ExternalInput").ap()
    bias = nc.dram_tensor("bias", [E], F32, kind="ExternalInput").ap()
    idx_out = nc.dram_tensor("idx", [tpc, K], U32, kind="ExternalOutput").ap()
    wts_out = nc.dram_tensor("wts", [tpc, K], F32, kind="ExternalOutput").ap()

    with tile.TileContext(nc) as tc, ExitStack() as ctx:
        const = ctx.enter_context(tc.tile_pool(name="const", bufs=1))
        htp = ctx.enter_context(tc.tile_pool(name="htp", bufs=2))
        psum = ctx.enter_context(tc.tile_pool(name="psum", bufs=4, space="PSUM"))
        work = ctx.enter_context(tc.tile_pool(name="work", bufs=3))
        small = ctx.enter_context(tc.tile_pool(name="small", bufs=4))
        stage = ctx.enter_context(tc.tile_pool(name="stage", bufs=2))

        # Router weight (transposed on host): resident in SBUF for the whole
        # kernel. Split into pieces so the first matmuls can start before
        # the full load lands.
        NWP = 4  # weight pieces

        def alloc_weight(dt_, name):
            return [
                const.tile(
                    [P, KT // NWP, E], dt_, tag=f"{name}{i}", name=f"{name}{i}"
                )
                for i in range(NWP)
            ]

        def load_weight_piece(ap, tiles, i):
            view = ap.rearrange("(k p) e -> p k e", p=P)  # [128, 32, 256]
            nc.sync.dma_start(
                out=tiles[i], in_=view[:, i * (KT // NWP):(i + 1) * (KT // NWP), :]
            )

        if fp16:
            wth_sb = alloc_weight(F16, "wth")
            wtl_sb = alloc_weight(F16, "wtl")
            # hi piece 0 first: the A-phase (hi.hi) matmuls of chunk 0 can
            # start as soon as it plus the first hth half arrive; everything
            # else streams in behind.
            load_weight_piece(wth, wth_sb, 0)
        else:
            wt_sb = alloc_weight(F32, "wt")
            load_weight_piece(wt, wt_sb, 0)

        bias_sb = const.tile([P, E], F32, tag="bias")
        bias_bcast = bass.AP(
            tensor=bias.tensor, offset=bias.offset, ap=[[0, P]] + list(bias.ap)
        )
        # issued on gpsimd (SWDGE) so it doesn't sit ahead of the critical
        # first weight/hidden pieces in the HWDGE FIFO; not needed until the
        # first sigmoid ~15us in

        SG = min(4, nt)  # tiles per output-stage group
        idx_out_v = idx_out.rearrange("(t p) r -> p t r", p=P)
        wts_out_v = wts_out.rearrange("(t p) r -> p t r", p=P)
        idx_stage = None
        wts_stage = None

        if fp16:
            hth_view = hth.rearrange("(k p) t -> p k t", p=P)
            htl_view = htl.rearrange("(k p) t -> p k t", p=P)
        else:
            ht_view = ht.rearrange("(k p) t -> p k t", p=P)  # [128, 32, tpc]

        # chunk widths (in token tiles): 2 except the final two chunks, which
        # are single-tile so the kernel tail (last tile's DVE chain after the
        # last matmul) is as short as possible
        if nt >= 4:
            widths = [2] * (nt // 2 - 1) + [1, 1]
        else:
            widths = [1] * nt
        starts = [sum(widths[:i]) for i in range(len(widths))]
        n_chunks = len(widths)
        for ci in range(n_chunks * repeat):
            c = ci % n_chunks
            CW = widths[c]
            t0 = starts[c] * P
            if fp16:
                hth_t = htp.tile([P, KT, CW * P], F16, tag="hth", name="hth_t")
                htl_t = htp.tile([P, KT, CW * P], F16, tag="htl", name="htl_t")
                # k-halves so low-k matmuls start after 1MB each; hi parts
                # first (A-phase runs before D-phase)
                for half in range(2):
                    ks = slice(half * (KT // 2), (half + 1) * (KT // 2))
                    nc.sync.dma_start(
                        out=hth_t[:, ks, :], in_=hth_view[:, ks, t0 : t0 + CW * P]
                    )
                    if ci == 0:
                        load_weight_piece(wth, wth_sb, 1 + half)
                if ci == 0:
                    load_weight_piece(wth, wth_sb, 3)
                    load_weight_piece(wtl, wtl_sb, 0)
                for half in range(2):
                    ks = slice(half * (KT // 2), (half + 1) * (KT // 2))
                    nc.sync.dma_start(
                        out=htl_t[:, ks, :], in_=htl_view[:, ks, t0 : t0 + CW * P]
                    )
                    if ci == 0:
                        load_weight_piece(wtl, wtl_sb, 1 + half)
                if ci == 0:
                    load_weight_piece(wtl, wtl_sb, 3)
                    nc.gpsimd.dma_start(out=bias_sb, in_=bias_bcast)
            else:
                ht_t = htp.tile([P, KT, CW * P], F32, tag="ht", name="ht_t")
                # two k-halves so PE can start after 2MB instead of 4MB
                nc.sync.dma_start(
                    out=ht_t[:, : KT // 2, :],
                    in_=ht_view[:, : KT // 2, t0 : t0 + CW * P],
                )
                nc.sync.dma_start(
                    out=ht_t[:, KT // 2 :, :],
                    in_=ht_view[:, KT // 2 :, t0 : t0 + CW * P],
                )
                if ci == 0:
                    for i in range(1, NWP):
                        load_weight_piece(wt, wt_sb, i)
                    nc.gpsimd.dma_start(out=bias_sb, in_=bias_bcast)
            for tt in range(CW):
                ti = starts[c] + tt
                tsl = slice(tt * P, (tt + 1) * P)
                if fp16:
                    ps_a = psum.tile([P, E], F32, tag="psa")   # hi.hi
                    ps_d = psum.tile([P, E], F32, tag="psd")   # hi.lo' + lo'.hi
                    # A phase first: only needs the hi tensors, so chunk 0's
                    # matmuls start after ~1.5MB of DMA instead of ~3MB
                    for k in range(KT):
                        wpi, wps = k // (KT // 4), k % (KT // 4)
                        nc.tensor.matmul(
                            ps_a,
                            lhsT=hth_t[:, k, tsl],
                            rhs=wth_sb[wpi][:, wps, :],
                            start=(k == 0),
                            stop=(k == KT - 1),
                        )
                    for k in range(KT):
                        wpi, wps = k // (KT // 4), k % (KT // 4)
                        nc.tensor.matmul(
                            ps_d,
                            lhsT=hth_t[:, k, tsl],
                            rhs=wtl_sb[wpi][:, wps, :],
                            start=(k == 0),
                            stop=False,
                        )
                        nc.tensor.matmul(
                            ps_d,
                            lhsT=htl_t[:, k, tsl],
                            rhs=wth_sb[wpi][:, wps, :],
                            start=False,
                            stop=(k == KT - 1),
                        )
                    # logits = ps_a + ps_d * 2^-12  (lo parts pre-scaled 2^12)
                    # scale on ACT (idle), add on DVE: each reads one PSUM
                    dsc = work.tile([P, E], F32, tag="dsc")
                    nc.scalar.activation(
                        dsc, ps_d, mybir.ActivationFunctionType.Copy,
                        scale=LO_INV,
                    )
                    logits = work.tile([P, E], F32, tag="logits")
                    nc.vector.tensor_add(logits, dsc, ps_a)
                    sig_in = logits
                else:
                    ps = psum.tile([P, E], F32, tag="ps")
                    for k in range(KT):
                        nc.tensor.matmul(
                            ps,
                            lhsT=ht_t[:, k, tsl],
                            rhs=wt_sb[k // (KT // 4)][:, k % (KT // 4), :],
                            start=(k == 0),
                            stop=(k == KT - 1),
                        )
                    sig_in = ps

                # scores = sigmoid(logits)  (also evicts PSUM -> SBUF)
                scores = work.tile([P, E], F32, tag="scores")
                nc.scalar.activation(
                    scores, sig_in, mybir.ActivationFunctionType.Sigmoid
                )
                # biased = scores + e_score_correction_bias
                biased = work.tile([P, E], F32, tag="biased")
                nc.vector.tensor_add(biased, scores, bias_sb)

                bg = biased.rearrange("p (g e) -> p g e", g=G)
                # group score = sum of top-2 biased scores within each group
                m1 = small.tile([P, G], F32, tag="m1")
                nc.vector.tensor_reduce(m1, bg, axis=mybir.AxisListType.X, op=ALU.max)
                b2 = work.tile([P, E], F32, tag="b2")
                nc.vector.match_replace(
                    out=b2, in_to_replace=m1, in_values=biased, imm_value=NEG
                )
                m2 = small.tile([P, G], F32, tag="m2")
                nc.vector.tensor_reduce(
                    m2, b2.rearrange("p (g e) -> p g e", g=G),
                    axis=mybir.AxisListType.X, op=ALU.max,
                )
                gs = small.tile([P, G], F32, tag="gs")
                nc.vector.tensor_add(gs, m1, m2)
                # top-4 groups: t4 = 4th largest group score; mask the rest
                g8 = small.tile([P, 8], F32, tag="g8")
                nc.vector.max(out=g8, in_=gs)
                pen = small.tile([P, G], F32, tag="pen")
                nc.vector.tensor_scalar(
                    pen, gs, g8[:, TOPK_GROUP - 1 : TOPK_GROUP], None, op0=ALU.is_lt
                )
                # mb = biased - 1e30 * (group not allowed)
                mb = work.tile([P, E], F32, tag="mb")
                nc.vector.scalar_tensor_tensor(
                    out=mb.rearrange("p (g e) -> p g e", g=G),
                    in0=pen.unsqueeze(-1).to_broadcast([P, G, EPG]),
                    scalar=NEG,
                    in1=bg,
                    op0=ALU.mult,
                    op1=ALU.add,
                )
                # top-8 experts by biased score (descending, ties -> low idx)
                v8 = small.tile([P, K], F32, tag="v8")
                nc.vector.max(out=v8, in_=mb)
                i8 = small.tile([P, K], U32, tag="i8")
                nc.vector.max_index(out=i8, in_max=v8, in_values=mb)

                # recover the UNbiased scores at those 8 positions:
                # mark positions via match_replace diff, pull their scores,
                # then re-order score-sorted results into biased-sorted order
                # by matching indices (positions are unique, so this is exact).
                dead = work.tile([P, E], F32, tag="dead")
                nc.vector.match_replace(
                    out=dead, in_to_replace=v8, in_values=mb, imm_value=NEG
                )
                dm = work.tile([P, E], F32, tag="dm")
                nc.vector.tensor_tensor(dm, mb, dead, op=ALU.not_equal)
                ssel = work.tile([P, E], F32, tag="ssel")
                nc.vector.tensor_mul(ssel, dm, scores)
                ws = small.tile([P, K], F32, tag="ws")
                nc.vector.max(out=ws, in_=ssel)
                iws = small.tile([P, K], U32, tag="iws")
                nc.vector.max_index(out=iws, in_max=ws, in_values=ssel)
                if8 = small.tile([P, K], F32, tag="if8")
                nc.vector.tensor_copy(if8, i8)
                if8s = small.tile([P, K], F32, tag="if8s")
                nc.vector.tensor_copy(if8s, iws)
                eq = small.tile([P, K, K], F32, tag="eq")
                nc.vector.tensor_tensor(
                    eq,
                    if8.unsqueeze(-1).to_broadcast([P, K, K]),
                    if8s.unsqueeze(1).to_broadcast([P, K, K]),
                    op=ALU.is_equal,
                )
                t8 = small.tile([P, K, K], F32, tag="t8")
                nc.vector.tensor_tensor(
                    t8, eq, ws.unsqueeze(1).to_broadcast([P, K, K]), op=ALU.mult
                )
                w8 = small.tile([P, K], F32, tag="w8")
                nc.vector.tensor_reduce(w8, t8, axis=mybir.AxisListType.X, op=ALU.add)

                # normalize and scale
                s8 = small.tile([P, 1], F32, tag="s8")
                nc.vector.tensor_reduce(s8, w8, axis=mybir.AxisListType.X, op=ALU.add)
                rec = small.tile([P, 1], F32, tag="rec")
                nc.vector.reciprocal(rec, s8)
                if ti % SG == 0:
                    idx_stage = stage.tile([P, SG, K], U32, tag="idxs", name="idxs")
                    wts_stage = stage.tile([P, SG, K], F32, tag="wtss", name="wtss")
                nc.vector.tensor_scalar(
                    wts_stage[:, ti % SG, :], w8, rec, ROUTED_SCALING,
                    op0=ALU.mult, op1=ALU.mult,
                )
                nc.vector.tensor_copy(idx_stage[:, ti % SG, :], i8)
                if ti % SG == SG - 1:
                    g0 = ti - (SG - 1)
                    nc.sync.dma_start(
                        out=idx_out_v[:, g0 : g0 + SG, :], in_=idx_stage
                    )
                    nc.sync.dma_start(
                        out=wts_out_v[:, g0 : g0 + SG, :], in_=wts_stage
                    )

    nc.compile()
    return nc


_CACHE: dict = {}


def _get_program():
    if "nc" not in _CACHE:
        _CACHE["nc"] = build_program()
    return _CACHE["nc"]


def _hilo(a):
    """Split fp32 -> (hi fp16, lo fp16 * 2^12). a = hi + lo/2^12 to ~2^-24."""
    hi = a.astype(np.float16)
    lo = ((a - hi.astype(np.float32)) * LO_SCALE).astype(np.float16)
    return hi, lo


def make_in_maps(hidden_states, weight, e_score_correction_bias):
    hidden = np.ascontiguousarray(np.asarray(hidden_states, dtype=np.float32))
    w = np.asarray(weight, dtype=np.float32)
    b = np.ascontiguousarray(np.asarray(e_score_correction_bias, dtype=np.float32))
    wt = np.ascontiguousarray(w.T)  # [4096, 256]
    in_maps = []
    if PRECISION == "fp16x3":
        wth, wtl = _hilo(wt)
        for c in range(N_CORES):
            sl = hidden[c * TPC : (c + 1) * TPC, :]     # [2048, 4096]
            ht = np.ascontiguousarray(sl.T)             # [4096, 2048]
            hth, htl = _hilo(ht)
            in_maps.append(
                {"hth": hth, "htl": htl, "wth": wth, "wtl": wtl, "bias": b}
            )
    else:
        for c in range(N_CORES):
            sl = hidden[c * TPC : (c + 1) * TPC, :]     # [2048, 4096]
            ht = np.ascontiguousarray(sl.T)             # [4096, 2048]
            in_maps.append({"ht": ht, "wt": wt, "bias": b})
    return in_maps


def kernel(hidden_states, weight, e_score_correction_bias):
    nc = _get_program()
    in_maps = make_in_maps(hidden_states, weight, e_score_correction_bias)
    res = run_bass_kernel_spmd(nc, in_maps, core_ids=list(range(N_CORES)))
    idx = np.concatenate(
        [res.results[c]["idx"].view(np.int32) for c in range(N_CORES)], axis=0
    )
    wts = np.concatenate(
        [res.results[c]["wts"] for c in range(N_CORES)], axis=0
    )
    return idx, wts


# revision 34
# speedup vs baseline: 1.0091x; 1.0091x over previous
"""NemotronH top-k MoE router on 8 Trainium2 NeuronCores (Bass/Tile).

Data-parallel over tokens: each of the 8 cores gets 2048 tokens.
Per core:
  - logits[128tok, 256e] = hidden @ weight.T at fp32-equivalent precision
    via an fp16 hi/lo decomposition (error ~2^-24, at fp32's own rounding
    noise) running at 3 PE cycles/row instead of plain fp32's 4, emitted
    as 2 matmuls per k-tile: one N=512 against [w_hi | w_lo'] computing
    the hi.hi and hi.lo' terms at once, one N=256 for lo'.hi
  - sigmoid (ACT)
  - DeepSeek-V3 style grouped top-k (noaux_tc) entirely with the DVE's
    native max/max_index/match_replace ops (ties resolve lowest-index
    first, exactly matching jax.lax.top_k)

Host side only reshapes/transposes/splits inputs (sharding prep) and
gathers outputs; all routing math runs on device.
"""

import sys
import numpy as np
from contextlib import ExitStack

for _p in ("/opt/trn_rl_repo", "/opt/pypackages"):
    if _p not in sys.path:
        sys.path.append(_p)

import concourse.bass as bass
import concourse.bacc as bacc
import concourse.tile as tile
import concourse.mybir as mybir
from concourse.bass_utils import run_bass_kernel_spmd

F32 = mybir.dt.float32
F16 = mybir.dt.float16
U32 = mybir.dt.uint32
ALU = mybir.AluOpType

# GEMM precision scheme:
#  "fp32"  : plain fp32 matmuls (4 cycles/row on the PE)
#  "fp16x3": x = hi + lo (fp16 hi, fp16 lo scaled by 2^12), w likewise;
#            logits = hi.hi + (hi.lo' + lo'.hi) * 2^-12, dropping the
#            lo.lo term (~2^-24 relative — at fp32's own noise floor).
#            3 matmuls at 1 cycle/row = 3 cycles/row total.
PRECISION = "fp16x3"
LO_SCALE = 4096.0          # 2^12
LO_INV = 1.0 / LO_SCALE

N_CORES = 8
TOKENS = 16384
HIDDEN = 4096
E = 256          # experts
G = 8            # groups
EPG = E // G     # experts per group (32)
TOPK_GROUP = 4
K = 8            # top-k experts
P = 128          # partitions
TPC = TOKENS // N_CORES   # tokens per core (2048)
KT = HIDDEN // P          # k tiles (32)
CHUNK = 2                 # token tiles per hidden DMA chunk
NEG = -1.0e30
ROUTED_SCALING = 2.5


def build_program(tpc: int = TPC, repeat: int = 1):
    """Build the SPMD Bass program (same on all cores).

    repeat > 1 re-runs the whole pipeline over the same data inside one
    NEFF — used only for wall-clock timing calibration (delta method).
    """
    nt = tpc // P  # token tiles per core
    nc = bacc.Bacc(
        "TRN2", target_bir_lowering=False, debug=False, num_devices=N_CORES
    )
    fp16 = PRECISION == "fp16x3"
    if fp16:
        hth = nc.dram_tensor("hth", [HIDDEN, tpc], F16, kind="ExternalInput").ap()
        htl = nc.dram_tensor("htl", [HIDDEN, tpc], F16, kind="ExternalInput").ap()
        # wc = [weightT_hi | weightT_lo*2^12] concatenated on the expert dim:
        # one N=512 matmul computes the hi.hi term AND the hi.lo cross term
        wc = nc.dram_tensor("wc", [HIDDEN, 2 * E], F16, kind="ExternalInput").ap()
    else:
        ht = nc.dram_tensor("ht", [HIDDEN, tpc], F32, kind="ExternalInput").ap()
        wt = nc.dram_tensor("wt", [HIDDEN, E], F32, kind="ExternalInput").ap()
    bias = nc.dram_tensor("bias", [E], F32, kind="ExternalInput").ap()
    idx_out = nc.dram_tensor("idx", [tpc, K], U32, kind="ExternalOutput").ap()
    wts_out = nc.dram_tensor("wts", [tpc, K], F32, kind="ExternalOutput").ap()

    with tile.TileContext(nc) as tc, ExitStack() as ctx:
        const = ctx.enter_context(tc.tile_pool(name="const", bufs=1))
        htp = ctx.enter_context(tc.tile_pool(name="htp", bufs=2))
        psum = ctx.enter_context(tc.tile_pool(name="psum", bufs=4, space="PSUM"))
        work = ctx.enter_context(tc.tile_pool(name="work", bufs=3))
        small = ctx.enter_context(tc.tile_pool(name="small", bufs=4))
        stage = ctx.enter_context(tc.tile_pool(name="stage", bufs=2))

        # Router weight (transposed on host): resident in SBUF for the whole
        # kernel. Split into pieces so the first matmuls can start before
        # the full load lands.
        NWP = 4  # weight pieces

        def alloc_weight(dt_, name):
            return [
                const.tile(
                    [P, KT // NWP, E], dt_, tag=f"{name}{i}", name=f"{name}{i}"
                )
                for i in range(NWP)
            ]

        def load_weight_piece(ap, tiles, i):
            view = ap.rearrange("(k p) e -> p k e", p=P)  # [128, 32, 256]
            nc.sync.dma_start(
                out=tiles[i], in_=view[:, i * (KT // NWP):(i + 1) * (KT // NWP), :]
            )

        NWPC = 8  # wc pieces (0.5MB each)
        if fp16:
            wc_sb = [
                const.tile(
                    [P, KT // NWPC, 2 * E], F16, tag=f"wc{i}", name=f"wc{i}"
                )
                for i in range(NWPC)
            ]
            wc_view = wc.rearrange("(k p) e -> p k e", p=P)  # [128, 32, 512]

            def load_wc_piece(i):
                nc.sync.dma_start(
                    out=wc_sb[i],
                    in_=wc_view[:, i * (KT // NWPC):(i + 1) * (KT // NWPC), :],
                )

            # piece 0 first: chunk 0's first matmuls start as soon as it plus
            # the first hth quarter arrive; the rest stream in behind.
            load_wc_piece(0)
        else:
            wt_sb = alloc_weight(F32, "wt")
            load_weight_piece(wt, wt_sb, 0)

        bias_sb = const.tile([P, E], F32, tag="bias")
        bias_bcast = bass.AP(
            tensor=bias.tensor, offset=bias.offset, ap=[[0, P]] + list(bias.ap)
        )
        # issued on gpsimd (SWDGE) so it doesn't sit ahead of the critical
        # first weight/hidden pieces in the HWDGE FIFO; not needed until the
        # first sigmoid ~15us in

        SG = min(4, nt)  # tiles per output-stage group
        idx_out_v = idx_out.rearrange("(t p) r -> p t r", p=P)
        wts_out_v = wts_out.rearrange("(t p) r -> p t r", p=P)
        idx_stage = None
        wts_stage = None

        if fp16:
            hth_view = hth.rearrange("(k p) t -> p k t", p=P)
            htl_view = htl.rearrange("(k p) t -> p k t", p=P)
        else:
            ht_view = ht.rearrange("(k p) t -> p k t", p=P)  # [128, 32, tpc]

        # chunk widths (in token tiles): 2 except the final two chunks, which
        # are single-tile so the kernel tail (last tile's DVE chain after the
        # last matmul) is as short as possible
        if nt >= 4:
            widths = [2] * (nt // 2 - 1) + [1, 1]
        else:
            widths = [1] * nt
        starts = [sum(widths[:i]) for i in range(len(widths))]
        n_chunks = len(widths)
        for ci in range(n_chunks * repeat):
            c = ci % n_chunks
            CW = widths[c]
            t0 = starts[c] * P
            if fp16:
                hth_t = htp.tile([P, KT, CW * P], F16, tag="hth", name="hth_t")
                htl_t = htp.tile([P, KT, CW * P], F16, tag="htl", name="htl_t")
                # hi parts first (AB-phase runs before C-phase); chunk 0 is
                # split into k-quarters so the first matmuls start after
                # ~0.5MB, and the remaining weight pieces stream in between
                nparts = 8 if ci == 0 else 2
                for part in range(nparts):
                    ks = slice(part * (KT // nparts), (part + 1) * (KT // nparts))
                    nc.sync.dma_start(
                        out=hth_t[:, ks, :], in_=hth_view[:, ks, t0 : t0 + CW * P]
                    )
                    if ci == 0 and part < 7:
                        load_wc_piece(1 + part)
                for part in range(nparts):
                    ks = slice(part * (KT // nparts), (part + 1) * (KT // nparts))
                    nc.sync.dma_start(
                        out=htl_t[:, ks, :], in_=htl_view[:, ks, t0 : t0 + CW * P]
                    )
                if ci == 0:
                    nc.gpsimd.dma_start(out=bias_sb, in_=bias_bcast)
            else:
                ht_t = htp.tile([P, KT, CW * P], F32, tag="ht", name="ht_t")
                # two k-halves so PE can start after 2MB instead of 4MB
                nc.sync.dma_start(
                    out=ht_t[:, : KT // 2, :],
                    in_=ht_view[:, : KT // 2, t0 : t0 + CW * P],
                )
                nc.sync.dma_start(
                    out=ht_t[:, KT // 2 :, :],
                    in_=ht_view[:, KT // 2 :, t0 : t0 + CW * P],
                )
                if ci == 0:
                    for i in range(1, NWP):
                        load_weight_piece(wt, wt_sb, i)
                    nc.gpsimd.dma_start(out=bias_sb, in_=bias_bcast)
            for tt in range(CW):
                ti = starts[c] + tt
                tsl = slice(tt * P, (tt + 1) * P)
                if fp16:
                    ps_ab = psum.tile([P, 2 * E], F32, tag="psab")  # [hi.hi | hi.lo']
                    ps_c = psum.tile([P, E], F32, tag="psc")        # lo'.hi
                    # AB phase first: only needs the hi hidden + wc, so chunk
                    # 0's matmuls start after ~1MB of DMA
                    for k in range(KT):
                        wpi, wps = k // (KT // NWPC), k % (KT // NWPC)
                        nc.tensor.matmul(
                            ps_ab,
                            lhsT=hth_t[:, k, tsl],
                            rhs=wc_sb[wpi][:, wps, :],
                            start=(k == 0),
                            stop=(k == KT - 1),
                        )
                    for k in range(KT):
                        wpi, wps = k // (KT // NWPC), k % (KT // NWPC)
                        nc.tensor.matmul(
                            ps_c,
                            lhsT=htl_t[:, k, tsl],
                            rhs=wc_sb[wpi][:, wps, :E],
                            start=(k == 0),
                            stop=(k == KT - 1),
                        )
                    # logits = A + (B + C) * 2^-12  (lo parts pre-scaled 2^12)
                    # each op reads at most one PSUM operand
                    dsc = work.tile([P, E], F32, tag="dsc")
                    nc.scalar.activation(
                        dsc, ps_c, mybir.ActivationFunctionType.Copy,
                        scale=LO_INV,
                    )
                    t1 = work.tile([P, E], F32, tag="t1")
                    nc.vector.scalar_tensor_tensor(
                        out=t1, in0=ps_ab[:, E:], scalar=LO_INV, in1=dsc,
                        op0=ALU.mult, op1=ALU.add,
                    )
                    logits = work.tile([P, E], F32, tag="logits")
                    nc.vector.tensor_add(logits, t1, ps_ab[:, :E])
                    sig_in = logits
                else:
                    ps = psum.tile([P, E], F32, tag="ps")
                    for k in range(KT):
                        nc.tensor.matmul(
                            ps,
                            lhsT=ht_t[:, k, tsl],
                            rhs=wt_sb[k // (KT // 4)][:, k % (KT // 4), :],
                            start=(k == 0),
                            stop=(k == KT - 1),
                        )
                    sig_in = ps

                # scores = sigmoid(logits)  (also evicts PSUM -> SBUF)
                scores = work.tile([P, E], F32, tag="scores")
                nc.scalar.activation(
                    scores, sig_in, mybir.ActivationFunctionType.Sigmoid
                )
                # biased = scores + e_score_correction_bias
                biased = work.tile([P, E], F32, tag="biased")
                nc.vector.tensor_add(biased, scores, bias_sb)

                bg = biased.rearrange("p (g e) -> p g e", g=G)
                # group score = sum of top-2 biased scores within each group
                m1 = small.tile([P, G], F32, tag="m1")
                nc.vector.tensor_reduce(m1, bg, axis=mybir.AxisListType.X, op=ALU.max)
                b2 = work.tile([P, E], F32, tag="b2")
                nc.vector.match_replace(
                    out=b2, in_to_replace=m1, in_values=biased, imm_value=NEG
                )
                m2 = small.tile([P, G], F32, tag="m2")
                nc.vector.tensor_reduce(
                    m2, b2.rearrange("p (g e) -> p g e", g=G),
                    axis=mybir.AxisListType.X, op=ALU.max,
                )
                gs = small.tile([P, G], F32, tag="gs")
                nc.vector.tensor_add(gs, m1, m2)
                # top-4 groups: t4 = 4th largest group score; mask the rest
                g8 = small.tile([P, 8], F32, tag="g8")
                nc.vector.max(out=g8, in_=gs)
                pen = small.tile([P, G], F32, tag="pen")
                nc.vector.tensor_scalar(
                    pen, gs, g8[:, TOPK_GROUP - 1 : TOPK_GROUP], None, op0=ALU.is_lt
                )
                # mb = biased - 1e30 * (group not allowed)
                mb = work.tile([P, E], F32, tag="mb")
                nc.vector.scalar_tensor_tensor(
                    out=mb.rearrange("p (g e) -> p g e", g=G),
                    in0=pen.unsqueeze(-1).to_broadcast([P, G, EPG]),
                    scalar=NEG,
                    in1=bg,
                    op0=ALU.mult,
                    op1=ALU.add,
                )
                # top-8 experts by biased score (descending, ties -> low idx)
                v8 = small.tile([P, K], F32, tag="v8")
                nc.vector.max(out=v8, in_=mb)
                i8 = small.tile([P, K], U32, tag="i8")
                nc.vector.max_index(out=i8, in_max=v8, in_values=mb)

                # recover the UNbiased scores at those 8 positions:
                # mark positions via match_replace diff, pull their scores,
                # then re-order score-sorted results into biased-sorted order
                # by matching indices (positions are unique, so this is exact).
                dead = work.tile([P, E], F32, tag="dead")
                nc.vector.match_replace(
                    out=dead, in_to_replace=v8, in_values=mb, imm_value=NEG
                )
                dm = work.tile([P, E], F32, tag="dm")
                nc.vector.tensor_tensor(dm, mb, dead, op=ALU.not_equal)
                ssel = work.tile([P, E], F32, tag="ssel")
                nc.vector.tensor_mul(ssel, dm, scores)
                ws = small.tile([P, K], F32, tag="ws")
                nc.vector.max(out=ws, in_=ssel)
                iws = small.tile([P, K], U32, tag="iws")
                nc.vector.max_index(out=iws, in_max=ws, in_values=ssel)
                if8 = small.tile([P, K], F32, tag="if8")
                nc.vector.tensor_copy(if8, i8)
                if8s = small.tile([P, K], F32, tag="if8s")
                nc.vector.tensor_copy(if8s, iws)
                eq = small.tile([P, K, K], F32, tag="eq")
                nc.vector.tensor_tensor(
                    eq,
                    if8.unsqueeze(-1).to_broadcast([P, K, K]),
                    if8s.unsqueeze(1).to_broadcast([P, K, K]),
                    op=ALU.is_equal,
                )
                t8 = small.tile([P, K, K], F32, tag="t8")
                nc.vector.tensor_tensor(
                    t8, eq, ws.unsqueeze(1).to_broadcast([P, K, K]), op=ALU.mult
                )
                w8 = small.tile([P, K], F32, tag="w8")
                nc.vector.tensor_reduce(w8, t8, axis=mybir.AxisListType.X, op=ALU.add)

                # normalize and scale
                s8 = small.tile([P, 1], F32, tag="s8")
                nc.vector.tensor_reduce(s8, w8, axis=mybir.AxisListType.X, op=ALU.add)
                rec = small.tile([P, 1], F32, tag="rec")
                nc.vector.reciprocal(rec, s8)
                if ti % SG == 0:
                    idx_stage = stage.tile([P, SG, K], U32, tag="idxs", name="idxs")
                    wts_stage = stage.tile([P, SG, K], F32, tag="wtss", name="wtss")
                nc.vector.tensor_scalar(
                    wts_stage[:, ti % SG, :], w8, rec, ROUTED_SCALING,
                    op0=ALU.mult, op1=ALU.mult,
                )
                nc.vector.tensor_copy(idx_stage[:, ti % SG, :], i8)
                if ti % SG == SG - 1:
                    g0 = ti - (SG - 1)
                    nc.sync.dma_start(
                        out=idx_out_v[:, g0 : g0 + SG, :], in_=idx_stage
                    )
                    nc.sync.dma_start(
                        out=wts_out_v[:, g0 : g0 + SG, :], in_=wts_stage
                    )

    nc.compile()
    return nc


_CACHE: dict = {}


def _get_program():
    if "nc" not in _CACHE:
        _CACHE["nc"] = build_program()
    return _CACHE["nc"]


def _hilo(a):
    """Split fp32 -> (hi fp16, lo fp16 * 2^12). a = hi + lo/2^12 to ~2^-24."""
    hi = a.astype(np.float16)
    lo = ((a - hi.astype(np.float32)) * LO_SCALE).astype(np.float16)
    return hi, lo


def make_in_maps(hidden_states, weight, e_score_correction_bias):
    hidden = np.ascontiguousarray(np.asarray(hidden_states, dtype=np.float32))
    w = np.asarray(weight, dtype=np.float32)
    b = np.ascontiguousarray(np.asarray(e_score_correction_bias, dtype=np.float32))
    wt = np.ascontiguousarray(w.T)  # [4096, 256]
    in_maps = []
    if PRECISION == "fp16x3":
        wth, wtl = _hilo(wt)
        wc = np.ascontiguousarray(np.concatenate([wth, wtl], axis=1))
        for c in range(N_CORES):
            sl = hidden[c * TPC : (c + 1) * TPC, :]     # [2048, 4096]
            ht = np.ascontiguousarray(sl.T)             # [4096, 2048]
            hth, htl = _hilo(ht)
            in_maps.append({"hth": hth, "htl": htl, "wc": wc, "bias": b})
    else:
        for c in range(N_CORES):
            sl = hidden[c * TPC : (c + 1) * TPC, :]     # [2048, 4096]
            ht = np.ascontiguousarray(sl.T)             # [4096, 2048]
            in_maps.append({"ht": ht, "wt": wt, "bias": b})
    return in_maps


def kernel(hidden_states, weight, e_score_correction_bias):
    nc = _get_program()
    in_maps = make_in_maps(hidden_states, weight, e_score_correction_bias)
    res = run_bass_kernel_spmd(nc, in_maps, core_ids=list(range(N_CORES)))
    idx = np.concatenate(
        [res.results[c]["idx"].view(np.int32) for c in range(N_CORES)], axis=0
    )
    wts = np.concatenate(
        [res.results[c]["wts"] for c in range(N_CORES)], axis=0
    )
    return idx, wts
